# revision 25
# baseline (speedup 1.0000x reference)
"""Biased attention Trainium2 kernel, SPMD over 8 NeuronCores.

Problem (per reference):
    sim  = q @ k^T / sqrt(64)                       [b,h,i,j]
    sim  = where(mask[b,j], sim, -fmax)
    sim -= taus[h] * attn_bias[b,i,j]
    out  = softmax(sim, axis=j) @ v                 [b,h,i,d]

Shapes: B=2, H=16, S=2048, D=64, fp32.

Sharding: batch*heads across 8 cores -> 4 (b,h) pairs per core, all with
the same batch b (core c handles b=c//4, heads 4*(c%4)..4*(c%4)+3), so
attn_bias/mask are batch-sharded and loaded once per core.

Key optimizations over the naive scheme:
  - Masked keys produce exactly-zero softmax weight (exp(-fmax)=0), so the
    key dimension is COMPACTED on host to the valid keys only (gather of
    k/v/bias rows), padded to a multiple of 128. For the fixed input this
    shrinks j from 2048 to 1152 -- nearly 2x less matmul, exp and bias
    work on device. The j-tile count is a compile-time constant; NEFFs
    are compiled (and cached) per observed tile count, so arbitrary masks
    still work.
  - All matmul operands are bf16 (1 cycle/row on PE, like f32r, but half
    the DMA/SBUF); accumulation stays fp32 in PSUM. Host pre-converts, so
    the device does zero ingest copies.
  - scores are computed TRANSPOSED: zT[j,i] = K Q^T, j on partitions, so
    softmax reductions run along the matmul contraction instead of
    needing a big transpose of the attention matrix.
  - compaction padding folds into the softmax exp as a per-partition bias
    (-1e30 on padded rows) on the ACT activation instruction.
  - the tau*attn_bias subtraction is load-balanced between PE
    (scaled-identity matmul accumulating -8*tau*biasT into the scores
    PSUM) and DVE (scalar_tensor_tensor (biasT * -8tau) + scores).
    1/sqrt(d)=1/8 folds into the exp scale.
  - V gets a ones-column appended so the softmax denominator falls out of
    the attention @ V matmul for free (row 64 of the [65,512] output).
  - output is un-transposed per 128-column block with PE transpose into
    the freed accumulator bank, then normalized with a per-partition
    reciprocal multiply and DMA'd out.
"""

import numpy as np
import ml_dtypes
from contextlib import ExitStack

import concourse.bass as bass
import concourse.tile as tile
from concourse import bacc, mybir
from concourse import bass_utils

F32 = mybir.dt.float32
BF16 = mybir.dt.bfloat16
Alu = mybir.AluOpType
Act = mybir.ActivationFunctionType

B, H, S, D = 2, 16, 2048, 64
N_CORES = 8
HPC = 4          # heads per core
NP = S // 512    # 4 i-panels of 512
BIG = 1.0e30

# Per-(pair, j % 9) engine for the tau*bias application: 'P' = PE
# scaled-identity matmul, 'D' = DVE scalar_tensor_tensor. Balanced so
# PE ~= DVE ~= ACT busy time.
ASSIGN = [
    "PDPPDDDDD",   # even heads
    "PDPPDDDDD",   # odd heads
]
# Max consecutive DVE-tiles sharing one ACT instruction.
DCHUNK = 1


def _build(jt, n_rep=1):
    """Build the SPMD kernel for jt j-tiles of 128 compacted keys."""
    nc = bacc.Bacc("TRN2", target_bir_lowering=False, debug=False,
                   num_devices=N_CORES)

    q_ap = nc.dram_tensor("qt", [2, 128, S], BF16, kind="ExternalInput").ap()
    k_ap = nc.dram_tensor("kt", [HPC, 128, jt * 128], BF16,
                          kind="ExternalInput").ap()
    v_ap = nc.dram_tensor("vp", [HPC, 128, jt * 65], BF16,
                          kind="ExternalInput").ap()
    n8tau_ap = nc.dram_tensor("n8tau", [128, HPC], F32,
                              kind="ExternalInput").ap()
    sci_ap = nc.dram_tensor("scaledI", [128, 128 * HPC], BF16,
                            kind="ExternalInput").ap()
    ident_ap = nc.dram_tensor("ident", [128, 128], F32,
                              kind="ExternalInput").ap()
    bias_ap = nc.dram_tensor("biasT", [jt * 128, S], BF16,
                             kind="ExternalInput").ap()
    out_ap = nc.dram_tensor("out", [HPC, NP, 128, 256], F32,
                            kind="ExternalOutput").ap()

    with tile.TileContext(nc) as tc:
        for _rep in range(n_rep):
            with ExitStack() as ctx:
                _body(ctx, tc, jt, q_ap, k_ap, v_ap, n8tau_ap, sci_ap,
                      ident_ap, bias_ap, out_ap)

    nc.compile()
    return nc


def _body(ctx, tc, jt, q_ap, k_ap, v_ap, n8tau_ap, sci_ap, ident_ap,
          bias_ap, out_ap):
    nc = tc.nc

    const = ctx.enter_context(tc.tile_pool(name="const", bufs=1))
    braw = ctx.enter_context(tc.tile_pool(name="braw", bufs=2 * jt + 2))
    zsb = ctx.enter_context(tc.tile_pool(name="zsb", bufs=3))
    epool = ctx.enter_context(tc.tile_pool(name="epool", bufs=5))
    dpool = ctx.enter_context(tc.tile_pool(name="dpool", bufs=3))
    zps = ctx.enter_context(tc.tile_pool(name="zps", bufs=3, space="PSUM"))
    ops = ctx.enter_context(tc.tile_pool(name="ops", bufs=1, space="PSUM"))

    # ---- constants (all host-prepared, DMA only) -----------------------
    # Spread prologue DMAs across idle queues so the first j-tile's inputs
    # (maskbias/n8tau/qt on SP, kt on ACT queue) land as early as possible.
    n8tau = const.tile([128, HPC], F32, tag="n8tau")
    nc.sync.dma_start(n8tau[:], n8tau_ap[:, :])

    # Q^T / K^T head-pair tiles [128, *]: even head on partitions 0-63,
    # odd head on 64-127 (host-packed, bf16). Split so the slices the
    # first few j-tiles need arrive first.
    qtr = []
    ktr = []
    for pair in range(2):
        qt = const.tile([128, S], BF16, tag=f"qtr{pair}")
        nc.sync.dma_start(qt[:, 0:1024], q_ap[pair, :, 0:1024])
        qtr.append(qt)
    for h in range(HPC):
        kt = const.tile([128, jt * 128], BF16, tag=f"ktr{h}")
        nc.scalar.dma_start(kt[:, 0:256], k_ap[h, :, 0:256])
        ktr.append(kt)
    for pair in range(2):
        nc.sync.dma_start(qtr[pair][:, 1024:S], q_ap[pair, :, 1024:S])
    for h in range(HPC):
        nc.scalar.dma_start(ktr[h][:, 256:jt * 128],
                            k_ap[h, :, 256:jt * 128])

    scaledI = const.tile([128, 128 * HPC], BF16, tag="scaledI")
    nc.gpsimd.dma_start(scaledI[:], sci_ap[:, :])

    # V with ones column, host-premarshalled [128, jt*65] per head
    vaug = []
    for h in range(HPC):
        va = const.tile([128, jt * 65], BF16, tag=f"vaug{h}")
        nc.gpsimd.dma_start(va[:], v_ap[h])
        vaug.append(va)
    ident = const.tile([128, 128], F32, tag="ident")
    nc.sync.dma_start(ident[:], ident_ap[:, :])

    # ---- main loops ----------------------------------------------------
    # Drains are emitted one j-iteration into the NEXT (P, pair) phase so
    # the next phase's first stt/exp aren't queued behind the drain's DVE
    # work at phase boundaries.
    def make_drain(P2, h, o):
        def drain(half):
            P = 2 * P2 + half
            ob = dpool.tile([65, 512], F32, tag="ob", name=f"ob_{h}_{P}")
            nc.vector.tensor_copy(ob[:], o[half][0:65, :])
            for c in range(4):
                nc.tensor.transpose(o[half][:, c * 65:(c + 1) * 65],
                                    ob[:, c * 128:(c + 1) * 128],
                                    ident[0:65, 0:65])
            oc = o[half][:, 0:260].rearrange("p (c x) -> p c x", x=65)
            rec = dpool.tile([128, 4], F32, tag="rec", name=f"rec_{h}_{P}")
            nc.vector.reciprocal(rec[:], oc[:, :, 64])
            ostage = dpool.tile([128, 256], F32, tag="ostage",
                                name=f"ostage_{h}_{P}")
            nc.vector.tensor_tensor(
                ostage[:].rearrange("p (c x) -> p c x", x=64),
                oc[:, :, 0:64],
                rec[:].broadcast_to((128, 4, 64)),
                op=Alu.mult)
            nc.sync.dma_start(out_ap[h, P], ostage[:])
        return drain

    # Each phase processes ONE head over a pair of i-panels (zp holds
    # [j, 1024] = two 512-wide panels), so the tau*bias stt is a single
    # [128,1024] instruction with a per-partition scalar, and bias tiles
    # are reused across the 4 heads of a panel-pair.
    #
    # The key-padding mask is baked into the bias padding rows (+1e30 on
    # host): -8*tau*1e30 drives the exp to exactly 0, so no per-j ACT
    # bias vector is needed and runs of consecutive DVE-tiles share one
    # larger ACT instruction (fewer fixed-cost instruction inits on the
    # bottleneck engine).
    pending = []
    av_pending = []

    def emit_av():
        if av_pending:
            av_pending.pop(0)()

    # split each phase's j range into single-P tiles and D-runs
    def plan(jt, row):
        groups = []
        j = 0
        while j < jt:
            if ASSIGN[row][j % 9] == "P":
                groups.append(("P", [j]))
                j += 1
            else:
                run = [j]
                j += 1
                while (j < jt and ASSIGN[row][j % 9] == "D"
                       and len(run) < DCHUNK):
                    run.append(j)
                    j += 1
                groups.append(("D", run))
        return groups

    for P2 in range(NP // 2):
        isl = slice(P2 * 1024, (P2 + 1) * 1024)
        bcache = [None] * jt
        for j in range(jt):
            bT = braw.tile([128, 1024], BF16, tag="bT", name=f"bT_{P2}_{j}")
            nc.gpsimd.dma_start(bT[:], bias_ap[j * 128:(j + 1) * 128, isl])
            bcache[j] = bT

        for h in range(HPC):
            pair = h // 2
            o = [ops.tile([128, 512], F32, tag=f"o{half}",
                          name=f"o{half}_{P2}_{h}")
                 for half in range(2)]
            for kind, js in plan(jt, h % 2):
                use_pe = kind == "P"
                n = len(js)
                if not use_pe:
                    zs = zsb.tile([128, 1024 * n], F32, tag=f"zs{n}",
                                  name=f"zs_{P2}_{h}_{js[0]}")
                et = epool.tile([128, 1024 * n], BF16, tag=f"et{n}",
                                name=f"et_{P2}_{h}_{js[0]}")
                for gi, j in enumerate(js):
                    jsl = slice(j * 128, (j + 1) * 128)
                    bT = bcache[j]
                    zp = zps.tile([128, 1024], F32, tag="zp",
                                  name=f"zp_{P2}_{h}_{j}")
                    for half in range(2):
                        zsl = slice(half * 512, (half + 1) * 512)
                        qsl = slice(P2 * 1024 + half * 512,
                                    P2 * 1024 + (half + 1) * 512)
                        nc.tensor.matmul(zp[:, zsl],
                                         lhsT=ktr[h][:, jsl],
                                         rhs=qtr[pair][:, qsl],
                                         start=True, stop=not use_pe)
                        if use_pe:
                            nc.tensor.matmul(
                                zp[:, zsl],
                                lhsT=scaledI[:, h * 128:(h + 1) * 128],
                                rhs=bT[:, zsl], start=False, stop=True,
                                skip_group_check=True)
                    if not use_pe:
                        nc.vector.scalar_tensor_tensor(
                            zs[:, gi * 1024:(gi + 1) * 1024], in0=bT[:],
                            scalar=n8tau[:, h:h + 1],
                            in1=zp[:], op0=Alu.mult, op1=Alu.add)
                if use_pe:
                    nc.scalar.activation(et[:], zp[:], Act.Exp, scale=0.125)
                else:
                    nc.scalar.activation(et[:], zs[:], Act.Exp, scale=0.125)

                for gi, j in enumerate(js):
                    def av(o=o, h=h, et=et, gi=gi, j=j):
                        for half in range(2):
                            nc.tensor.matmul(
                                o[half][0:65, :],
                                lhsT=vaug[h][:, j * 65:(j + 1) * 65],
                                rhs=et[:, gi * 1024 + half * 512:
                                       gi * 1024 + (half + 1) * 512],
                                start=(j == 0), stop=(j == jt - 1))
                    av_pending.append(av)
                    if len(av_pending) > 1:
                        emit_av()
                if pending and js[0] in (0, 1):
                    pending.pop(0)()

            drain = make_drain(P2, h, o)
            pending.extend([lambda d=drain: d(0), lambda d=drain: d(1)])

    emit_av()
    for fn in pending:
        fn()


_NC_CACHE = {}


def _get_nc(jt):
    if jt not in _NC_CACHE:
        _NC_CACHE[jt] = _build(jt)
    return _NC_CACHE[jt]


def _prep_core(q, k, v, taus, attn_bias, valid, jt, b, h0):
    """Host-side marshalling for one core: compaction gather + bf16 pack."""
    J = jt * 128
    nv = len(valid)

    # q: [4, S, D] -> per pair [128, S] (head d-rows stacked), bf16
    qt = np.empty((2, 128, S), dtype=ml_dtypes.bfloat16)
    for pair in range(2):
        qt[pair, 0:64] = q[b, h0 + 2 * pair].T
        qt[pair, 64:128] = q[b, h0 + 2 * pair + 1].T

    # k: gather valid rows, pad to J. One [128, J] stationary per head,
    # zero-padded to full K=128: rows (h%2)*64..+64 carry k^T, the rest
    # are zero so the other head's q rows in the shared moving tensor are
    # multiplied away. (K=128 stationaries are ~1.6x faster per matmul on
    # real HW than K=64 ones.)
    kt = np.zeros((HPC, 128, J), dtype=ml_dtypes.bfloat16)
    for h in range(HPC):
        r0 = (h % 2) * 64
        kt[h, r0:r0 + 64, 0:nv] = k[b, h0 + h][valid].T

    # v: gather valid rows, pad, pack [128, jt, 65] with ones column
    vp = np.zeros((HPC, J, 65), dtype=np.float32)
    for h in range(HPC):
        vp[h, 0:nv, 0:64] = v[b, h0 + h][valid]
    vp[:, :, 64] = 1.0
    vp = np.ascontiguousarray(
        vp.reshape(HPC, jt, 128, 65).transpose(0, 2, 1, 3)
        .reshape(HPC, 128, jt * 65)).astype(ml_dtypes.bfloat16)

    # bias rows gathered: [J, S] bf16 (biasT = attn_bias[b].T).
    # Padding rows get +1e30: -8*tau*1e30 sends the exp argument to -inf
    # so padded keys get exactly-zero attention weight (for any tau > 0).
    biasT = np.full((J, S), 1e30, dtype=ml_dtypes.bfloat16)
    biasT[0:nv] = attn_bias[b].T[valid]

    # -8*tau broadcast + scaled identities
    t4 = taus[h0:h0 + HPC].astype(np.float32)
    n8tau = np.broadcast_to(-8.0 * t4, (128, HPC)).copy()
    sci = np.zeros((128, HPC * 128), dtype=ml_dtypes.bfloat16)
    eye = np.eye(128, dtype=np.float32)
    for h in range(HPC):
        sci[:, h * 128:(h + 1) * 128] = (-8.0 * t4[h]) * eye

    return {
        "qt": qt, "kt": kt, "vp": vp,
        "n8tau": n8tau, "scaledI": sci,
        "ident": eye, "biasT": biasT,
    }


def kernel(q, k, v, mask, taus, attn_bias):
    q = np.asarray(q, dtype=np.float32)
    k = np.asarray(k, dtype=np.float32)
    v = np.asarray(v, dtype=np.float32)
    mask = np.asarray(mask)
    taus = np.asarray(taus, dtype=np.float32)
    attn_bias = np.asarray(attn_bias, dtype=np.float32)

    valids = [np.where(mask[b])[0] for b in range(B)]
    max_nv = max(max(len(vv) for vv in valids), 1)
    jt = (max_nv + 127) // 128

    nc = _get_nc(jt)
    in_maps = []
    for c in range(N_CORES):
        b = c // 4
        h0 = (c % 4) * 4
        in_maps.append(
            _prep_core(q, k, v, taus, attn_bias, valids[b], jt, b, h0))

    res = bass_utils.run_bass_kernel_spmd(nc, in_maps,
                                          core_ids=list(range(N_CORES)))
    out = np.empty((B, H, S, D), dtype=np.float32)
    for c in range(N_CORES):
        b = c // 4
        h0 = (c % 4) * 4
        arr = res.results[c]["out"].reshape(HPC, NP, 128, 4, 64)
        # i = P*512 + c*128 + p  ->  [h, (P, c, p), d]
        out[b, h0:h0 + HPC] = arr.transpose(0, 1, 3, 2, 4).reshape(HPC, S, D)
    return out


if __name__ == "__main__":
    rng = np.random.default_rng(0)
    inputs = {
        "q": rng.standard_normal((B, H, S, D), dtype=np.float32),
        "k": rng.standard_normal((B, H, S, D), dtype=np.float32),
        "v": rng.standard_normal((B, H, S, D), dtype=np.float32),
        "mask": rng.random((B, S)) < 0.5,
        "taus": rng.random(H, dtype=np.float32),
        "attn_bias": rng.random((B, S, S), dtype=np.float32),
    }
    o = kernel(**inputs)
    print("out", o.shape, o.dtype, np.isfinite(o).all())
